# revision 1
# baseline (speedup 1.0000x reference)
"""Cube padding kernel for Trainium2 (Bass/Tile), 8-core SPMD.

Op: x [B=4, 6, C=64, H=128, W=128] f32 -> out [B, 6, C, H+2P, W+2P], P=2.
Each face's pad ring is gathered from neighboring faces (with flips /
transposes per the cube-net layout) and corners replicate the top/bottom
strip edge values.

Sharding: channel-parallel. C=64 is split into 8 chunks of 8 channels; every
core holds all 6 faces for its channel slice, so no cross-core traffic.

Per-core dataflow (per batch b):
  - DMA each face plane HBM -> SBUF tile O[f][h, c, 2:130] (the interior of
    the output rows; border columns are filled in place so the store back to
    HBM is one 528B-contiguous run per row).
  - PE transposes (exact pass-through via identity matmul) produce the
    transposed border data needed by the left/right/top/down faces.
  - DVE fills left/right border columns of O from other faces' SBUF tiles or
    PSUM transposes (partition-preserving copies).
  - Top/bottom 2-row strips (+corners) are assembled in small [16,136] tiles
    (partition = (r, c)) via DMA / DVE-from-PSUM + corner broadcasts, then
    stored as 528B rows.

All SBUF/PSUM access patterns keep the partition index in the leading AP dim
with stride == the tile's free-row size (plain partition-range slices): both
the simulator's shadow tracker and the DMA addr64 lowering are only reliable
for that form.
"""

import numpy as np

import concourse.bacc as bacc
import concourse.bass as bass
import concourse.mybir as mybir
from concourse import tile
from concourse.bass_utils import run_bass_kernel_spmd

P = 2
B, F, C, H, W = 4, 6, 64, 128, 128
NCORES = 8
CL = C // NCORES  # channels per core
HO, WO = H + 2 * P, W + 2 * P  # 132, 132
TW = 136  # strip tile row = full 544B slot row (32B-aligned)
FP32 = mybir.dt.float32

# face indices (order of unpacking in the reference: fb, fd, ff, fl, fr, ft)
BACK, DOWN, FRONT, LEFT, RIGHT, TOP = range(6)

# --- strip source tables -----------------------------------------------------
# TOP strips fill output rows 0..1 (r=0 is row 0), BOT strips rows 130..131
# (r=0 is row 130). Kinds:
#   ('rows', face, [i0, i1])  strip row r <- face row i_r          (DMA)
#   ('ptc',  face, [w0, w1])  strip row r <- face column w_r, transposed on
#                             PE with columns pre-ordered (w0, w1) so the
#                             PSUM result is evacuated by an identity-
#                             partition DVE copy.
TOP_SRC = {
    BACK: ("rows", TOP, [1, 0]),
    DOWN: ("rows", FRONT, [126, 127]),
    FRONT: ("rows", TOP, [126, 127]),
    LEFT: ("ptc", TOP, [0, 1]),
    RIGHT: ("ptc", TOP, [127, 126]),
    TOP: ("rows", BACK, [1, 0]),
}
BOT_SRC = {
    BACK: ("rows", DOWN, [127, 126]),
    DOWN: ("rows", BACK, [127, 126]),
    FRONT: ("rows", DOWN, [0, 1]),
    LEFT: ("ptc", DOWN, [1, 0]),
    RIGHT: ("ptc", DOWN, [126, 127]),
    TOP: ("rows", FRONT, [0, 1]),
}
# LFT fills O cols 0..1 (k=0 is col 0), RGT fills O cols 130..131. Kinds:
#   ('cols', face, [w0, w1])  col k <- face col w_k     (DVE from O[face])
#   ('pt',   face, [j0, j1])  col k <- PT[face][:, c, j_k] (DVE from PSUM)
LFT_SRC = {
    BACK: ("cols", RIGHT, [126, 127]),
    DOWN: ("pt", LEFT, [127, 126]),
    FRONT: ("cols", LEFT, [126, 127]),
    LEFT: ("cols", BACK, [126, 127]),
    RIGHT: ("cols", FRONT, [126, 127]),
    TOP: ("pt", LEFT, [0, 1]),
}
RGT_SRC = {
    BACK: ("cols", LEFT, [0, 1]),
    DOWN: ("pt", RIGHT, [126, 127]),
    FRONT: ("cols", RIGHT, [0, 1]),
    LEFT: ("cols", FRONT, [0, 1]),
    RIGHT: ("cols", BACK, [0, 1]),
    TOP: ("pt", RIGHT, [1, 0]),
}


def _pair_slice(idx):
    """Slice selecting [i0, i1] for adjacent pairs (ascending or descending)."""
    i0, i1 = idx
    assert abs(i1 - i0) == 1
    if i1 > i0:
        return slice(i0, i1 + 1)
    return slice(i0, (i1 - 1) if i1 > 0 else None, -1)


def build_kernel(nc, tc, xin, ident, yout, sim_safe=False, loop_n=None):
    # Every SBUF tile gets a unique tag with bufs=1: the whole working set
    # (~130KB/partition) fits in SBUF, so no slot recycling is needed and the
    # scheduler gets maximal reordering freedom.
    import contextlib

    with (
        tc.tile_pool(name="const", bufs=1) as const_pool,
        tc.tile_pool(name="io", bufs=1) as io_pool,
        tc.tile_pool(name="strips", bufs=1) as strip_pool,
        tc.tile_pool(name="psum", bufs=1, space="PSUM") as psum_pool,
    ):
        idt = const_pool.tile([128, 128], FP32, name="idt")
        nc.gpsimd.dma_start(idt[:, :], ident[:, :])

        # loop_n: timing-only mode — repeat the whole (idempotent) body N
        # times in a hardware loop so wall-clock/N isolates kernel time from
        # the ~100ms axon dispatch overhead.
        loop_ctx = tc.For_i(0, loop_n, 1) if loop_n else contextlib.nullcontext()
        with loop_ctx:
            _build_body(nc, tc, xin, idt, yout, io_pool, strip_pool,
                        psum_pool, sim_safe)


def _build_body(nc, tc, xin, idt, yout, io_pool, strip_pool, psum_pool,
                sim_safe):
        for b in range(B):
            # --- load all 6 face interiors into O tiles ---
            # The 8-col edge memsets cover the border cols' 32B sectors (the
            # sim's uninit tracker works at sector granularity) and overlap
            # the interior DMA region so program order is enforced via WAW.
            O = {}
            for f in range(F):
                O[f] = io_pool.tile(
                    [128, CL, WO], FP32, name=f"O{b}_{f}", tag=f"O{b}_{f}"
                )
                if sim_safe:
                    # only to satisfy the simulator's sector-granular uninit
                    # tracker; every byte is overwritten before being read
                    nc.gpsimd.memset(O[f][:, :, 0:8], 0.0)
                    nc.gpsimd.memset(O[f][:, :, WO - 8 : WO], 0.0)
                nc.sync.dma_start(
                    O[f][:, :, P : P + W], xin[b, f].transpose((1, 0, 2))
                )

            # --- PE transposes ---
            # Full-plane transposes of LEFT/RIGHT faces (per channel):
            # PT[f][p, c, j] = face[c, j, p]
            PT = {}
            for f in (LEFT, RIGHT):
                PT[f] = psum_pool.tile(
                    [128, CL, 128], FP32, name=f"PT{b}_{f}", tag=f"PT{f}"
                )
                for c in range(CL):
                    nc.tensor.transpose(
                        PT[f][:, c, :], O[f][:, c, P : P + W], idt[:, :]
                    )

            # Single-column transposes for the 'ptc' strips: pts[:, i, :] is
            # [CL part = c, 128] = one transposed source column, in consumer
            # row order (PE matmul and DVE both require partition base 0, so
            # the strip rows live in the free dim of [CL, 2, TW] tiles).
            pts = psum_pool.tile([CL, 8, 128], FP32, name=f"pts{b}", tag="pts", bufs=2)
            ptc_out = {}
            ptc_specs = [
                (TOP_SRC, "Ttop", LEFT),
                (TOP_SRC, "Ttop", RIGHT),
                (BOT_SRC, "Tbot", LEFT),
                (BOT_SRC, "Tbot", RIGHT),
            ]
            for i, (table, sname, f) in enumerate(ptc_specs):
                _, src_f, wpair = table[f]
                outs = []
                for r in range(2):
                    col = P + wpair[r]
                    nc.tensor.transpose(
                        pts[:, 2 * i + r, :],
                        O[src_f][:, :, col : col + 1].squeeze(),
                        idt[:, :],
                    )
                    outs.append(pts[:, 2 * i + r, :])
                ptc_out[(sname, f)] = outs

            for f in range(F):
                # --- left/right border columns of O (DVE, partition-preserving) ---
                for dst_col, table in ((0, LFT_SRC), (W + P, RGT_SRC)):
                    kind, src_f, idx = table[f][0], table[f][1], table[f][2]
                    if kind == "cols":
                        src = O[src_f][:, :, P + idx[0] : P + idx[1] + 1]
                    else:  # 'pt'
                        src = PT[src_f][:, :, _pair_slice(idx)]
                    nc.vector.tensor_copy(O[f][:, :, dst_col : dst_col + 2], src)

                # --- top/bottom strips ---
                # Rows 0,1 (and 130,131) are contiguous 264-element runs in
                # DRAM, so strip tiles are 132 wide (1056B rows, 32B-aligned).
                # rows+rows faces (0,1,2,5) assemble all 4 border rows in one
                # [16, 2, 132] tile (partition = pair*CL+c) -> ONE store;
                # ptc faces (3,4) use two [CL, 2, 132] tiles (partition = c,
                # PE/DVE need partition base 0) -> one store per pair.
                if TOP_SRC[f][0] == "rows":
                    TT = strip_pool.tile(
                        [2 * CL, 2, WO], FP32, name=f"TT{b}_{f}", tag=f"TT{b}_{f}"
                    )
                    if sim_safe:
                        nc.gpsimd.memset(TT[:, :, :], 0.0)
                    for pair, table in ((0, TOP_SRC), (1, BOT_SRC)):
                        _, src_f, idx = table[f]
                        nc.gpsimd.dma_start(
                            TT[pair * CL : (pair + 1) * CL, :, P : P + W],
                            xin[b, src_f][:, _pair_slice(idx), :],
                        )
                    nc.vector.tensor_copy(
                        TT[:, :, 0:P],
                        TT[:, :, P : P + 1].broadcast_to((2 * CL, 2, P)),
                    )
                    nc.vector.tensor_copy(
                        TT[:, :, P + W :],
                        TT[:, :, P + W - 1 : P + W].broadcast_to((2 * CL, 2, P)),
                    )
                    # one DMA for rows {0,1} and {130,131}: iterate
                    # (pair, c, row-pair run); both sides 3 dims, all positive
                    yb = yout[b, f]
                    dst = bass.AP(
                        yb.tensor,
                        yb.offset,
                        [[(H + P) * WO, 2], [HO * WO, CL], [1, 2 * WO]],
                    )
                    eng = nc.sync if f % 2 == 0 else nc.scalar
                    eng.dma_start(dst, TT.rearrange("p r w -> p (r w)"))
                else:
                    for si, (sname, table, rows0) in enumerate(
                        (("Ttop", TOP_SRC, 0), ("Tbot", BOT_SRC, P + H))
                    ):
                        Trc = strip_pool.tile(
                            [CL, 2, WO], FP32,
                            name=f"{sname}{b}_{f}", tag=f"{sname}{b}_{f}",
                        )
                        if sim_safe:
                            nc.gpsimd.memset(Trc[:, :, :], 0.0)
                        for r in range(2):
                            nc.vector.tensor_copy(
                                Trc[:, r, P : P + W], ptc_out[(sname, f)][r]
                            )
                        nc.vector.tensor_copy(
                            Trc[:, :, 0:P],
                            Trc[:, :, P : P + 1].broadcast_to((CL, 2, P)),
                        )
                        nc.vector.tensor_copy(
                            Trc[:, :, P + W :],
                            Trc[:, :, P + W - 1 : P + W].broadcast_to((CL, 2, P)),
                        )
                        dst = yout[b, f][:, rows0 : rows0 + P, :]
                        eng = nc.sync if (f + si) % 2 == 0 else nc.scalar
                        eng.dma_start(
                            dst.rearrange("c r w -> c (r w)"),
                            Trc.rearrange("c r w -> c (r w)"),
                        )

                # --- interior + left/right borders store (rows 2..129) ---
                nc.scalar.dma_start(
                    yout[b, f][:, P : P + H, :].transpose((1, 0, 2)), O[f][:, :, :]
                )


def build_nc(debug=False, detect_races=True, sim_safe=False, loop_n=None):
    nc = bacc.Bacc(
        "TRN2",
        target_bir_lowering=False,
        debug=debug,
        detect_race_conditions=detect_races,
    )
    xin = nc.dram_tensor("x", [B, F, CL, H, W], FP32, kind="ExternalInput")
    ident = nc.dram_tensor("ident", [128, 128], FP32, kind="ExternalInput")
    yout = nc.dram_tensor("y", [B, F, CL, HO, WO], FP32, kind="ExternalOutput")
    with tile.TileContext(nc) as tc:
        build_kernel(
            nc, tc, xin.ap(), ident.ap(), yout.ap(),
            sim_safe=sim_safe, loop_n=loop_n,
        )
    nc.compile()  # bacc passes (register allocation etc.) — required for NEFF
    return nc


_IDENT = np.eye(128, dtype=np.float32)


def kernel(x: np.ndarray) -> np.ndarray:
    x = np.asarray(x, dtype=np.float32)
    assert x.shape == (B, F, C, H, W), x.shape
    nc = build_nc()
    in_maps = [
        {
            "x": np.ascontiguousarray(x[:, :, k * CL : (k + 1) * CL]),
            "ident": _IDENT,
        }
        for k in range(NCORES)
    ]
    res = run_bass_kernel_spmd(nc, in_maps, core_ids=list(range(NCORES))).results
    return np.concatenate([res[k]["y"] for k in range(NCORES)], axis=2)

